# revision 41
# baseline (speedup 1.0000x reference)
"""CRF loss (2-state FSA) on 8 Trainium2 NeuronCores — transposed streaming.

Math: with y = exp(log_probs), the per-step denominator scores are linear in
y:  S0 = sum_c y[c]*U0[c];  S1 = sum_c y[c]*U1[c];  p = y[2]
where U0/U1 are the per-state softmax of den_scores mapped through the arc
table (O(L) host prep, like the scan masks and final-arc score). The 2-state
forward recurrence runs in REAL space as products of 2x2 matrices
  M_t = [[S0, S1], [p*e00, p*e11]]
composed on-device over chunks of LCH=2 steps (scaled by 32 per step against
fp32 underflow). Steps past input_len become 32*I. Raw chunk matrices ship
per half-eighth; the host takes logs in f64, folds the per-sequence chains
with LSE, and sums.

Layout: host ships lp TRANSPOSED as lpT[c, g] fp16 (half the f32 DMA bytes;
~64x less denominator rounding bias than bf16) with column order
g = i*128 + pi, where pi = (seq_local*16 + toff) is the scan partition and
i the within-partition step (t = toff*256 + i). Then:
  - exp on ACT gives y16T in the same layout;
  - S0/S1/p for scan column i = one tiny PE matmul y16T[:, i-block].T @ W
    (W columns: U0, U1, onehot(2), 0) accumulating straight into
    scan-layout PSUM — one [128, 4*NIH] drain per eighth;
  - numerator: PE broadcasts labels (ones[1,128].T @ labT) into PSUM
    (software-pipelined one chunk ahead), one fused DVE STT per 1024-column
    chunk computes (lab==iota_c)*lpT, and PE ones-matmuls accumulate the
    masked values into a [128,1] PSUM (exact fp16 gather, fp32 accumulate).

The 32-chunk DVE STT stream (~38 us) is the critical path: startup DMAs are
ordered so the first STT fires ASAP (lp/label prefixes before bulk loads),
S-matmuls for each eighth are deferred into the next eighth's numerator
stream so their exp-wait cannot head-of-line-block PE's label broadcasts,
and scan prep/compose (Pool) + chunk shipping overlap the stream per eighth
so the only tail is the numerator drain.

Sharding: data-parallel over batch; core k owns sequences [8k, 8k+8).
"""

import os
import sys

import numpy as np

for _p in ("/opt/trn_rl_repo", os.path.expanduser("~/.axon_site/_ro/trn_rl_repo")):
    if os.path.isdir(_p) and _p not in sys.path:
        sys.path.insert(0, _p)

import concourse.bacc as bacc
import concourse.bass as bass
import concourse.mybir as mybir
import concourse.tile as tile
from concourse.bass_utils import run_bass_kernel_spmd

F32 = mybir.dt.float32
FP16 = mybir.dt.float16
Alu = mybir.AluOpType
Act = mybir.ActivationFunctionType

L = 125
C = 128          # symbol classes
B, T = 64, 4096
NCORES = 8
BSH = B // NCORES            # sequences per core = 8
BT = BSH * T                 # positions per core = 32768
NI = BT // 128               # steps per scan partition = 256
NH = 8                       # half-quarters (DMA/exp granularity)
GH = BT // NH                # positions per half = 4096
NIH = NI // NH               # scan steps per half = 32
LCH = 2                      # scan chunk length (steps composed on device)
NCH = NI // LCH              # 128 chunk matrices per partition
NCHH = NIH // LCH            # chunk matrices per half = 16
SCALE = 32.0                 # per-step scaling against fp32 underflow
CK = 1024                    # numerator STT chunk positions
NCKH = GH // CK              # chunks per half = 4

# tiny fp16 consts: [iota_c | W(4 cols)]
TW = 5
# packed f32 consts: [w32 | w32c | e_b(2 cols)]
O_W32, O_W32C, O_EB = 0, NI, 2 * NI
CPK = 2 * NI + 2


def _build_program():
    nc = bacc.Bacc("TRN2", target_bir_lowering=False, debug=False)

    lpt_d = nc.dram_tensor("lpt", [128, BT], FP16, kind="ExternalInput")
    labt_d = nc.dram_tensor("labt", [1, BT], FP16, kind="ExternalInput")
    tw_d = nc.dram_tensor("tw", [128, TW], FP16, kind="ExternalInput")
    cpk_d = nc.dram_tensor("cpk", [128, CPK], F32, kind="ExternalInput")

    chunkraw_d = nc.dram_tensor("chunkraw", [128, 4, NCH], F32, kind="ExternalOutput")
    numpart_d = nc.dram_tensor("numpart", [128, 1], F32, kind="ExternalOutput")

    with tile.TileContext(nc) as tc:
        with (
            tc.tile_pool(name="const", bufs=1) as cpool,
            tc.tile_pool(name="small", bufs=1) as spool,
            tc.tile_pool(name="y16", bufs=2) as y_pool,
            tc.tile_pool(name="msk", bufs=4) as m_pool,
            tc.tile_pool(name="scan", bufs=1) as scan_pool,
            tc.tile_pool(name="psS", bufs=2, space=bass.MemorySpace.PSUM) as psS,
            tc.tile_pool(name="psL", bufs=2, space=bass.MemorySpace.PSUM) as psL,
            tc.tile_pool(name="psN", bufs=1, space=bass.MemorySpace.PSUM) as psN,
        ):
            # ------- startup loads, ordered for earliest first STT; all
            # load DMAs are issued up-front so later output stores can never
            # block them in the SP queue -------
            lpts = []
            for _hh in range(NH):
                _t = cpool.tile([128, GH], FP16, tag=f"lpt{_hh}")
                lpts.append(_t)
            nc.sync.dma_start(lpts[0][:, 0:CK], lpt_d.ap()[:, 0:CK])
            tw = cpool.tile([128, TW], FP16)
            nc.sync.dma_start(tw[:], tw_d.ap())
            labt_a = cpool.tile([1, 2 * GH], FP16)
            nc.sync.dma_start(labt_a[:], labt_d.ap()[:, 0 : 2 * GH])
            # warmup pieces sized so the DVE stream never starves
            PIECES = [(0, CK, 2 * CK), (0, 2 * CK, GH),
                      (1, 0, 2 * CK), (1, 2 * CK, GH)]
            for _hh, _a, _b in PIECES:
                nc.sync.dma_start(
                    lpts[_hh][:, _a:_b],
                    lpt_d.ap()[:, _hh * GH + _a : _hh * GH + _b],
                )
            nc.sync.dma_start(
                lpts[2][:, 0 : 2 * CK], lpt_d.ap()[:, 2 * GH : 2 * GH + 2 * CK]
            )
            cpk = cpool.tile([128, CPK], F32)
            nc.sync.dma_start(cpk[:], cpk_d.ap())
            labt = cpool.tile([1, BT], FP16)
            nc.sync.dma_start(labt[:], labt_d.ap())
            nc.sync.dma_start(
                lpts[2][:, 2 * CK : GH], lpt_d.ap()[:, 2 * GH + 2 * CK : 3 * GH]
            )
            for _hh in range(3, NH):
                nc.sync.dma_start(
                    lpts[_hh][:], lpt_d.ap()[:, _hh * GH : (_hh + 1) * GH]
                )

            iota_c = tw[:, 0:1]
            w4 = tw[:, 1:5]
            w32 = cpk[:, O_W32:O_W32 + NI]
            w32c = cpk[:, O_W32C:O_W32C + NI]
            e_b = cpk[:, O_EB:O_EB + 2]

            ones1 = cpool.tile([1, 128], FP16)
            nc.gpsimd.memset(ones1[:], 1.0)
            ones128 = cpool.tile([128, 1], FP16)
            nc.gpsimd.memset(ones128[:], 1.0)

            numacc_a = psN.tile([128, 1], F32, tag="na")
            numpart = spool.tile([128, 1], F32)
            nmm = [0]
            N_MM_TOTAL = NH * NCKH * (CK // 128)   # incl. deferred pool chunks

            def bcast(hh, h):
                lab_ps = psL.tile([128, CK], F32)
                off = hh * GH + h * CK
                lab_src = labt_a if off < 2 * GH else labt
                for z in range(CK // 512):
                    nc.tensor.matmul(
                        lab_ps[:, z * 512 : (z + 1) * 512],
                        ones1[:],
                        lab_src[:, off + z * 512 : off + (z + 1) * 512],
                        start=True, stop=True,
                    )
                return lab_ps

            lab_q = [bcast(0, 0)]

            def num_stream(hh, lpt, hooks=()):
                for h in range(NCKH):
                    if h + 1 < NCKH:
                        lab_q.append(bcast(hh, h + 1))
                    elif hh + 1 < NH:
                        lab_q.append(bcast(hh + 1, 0))
                    lab_ps = lab_q.pop(0)
                    msk = m_pool.tile([128, CK], FP16)
                    nc.vector.scalar_tensor_tensor(
                        msk[:], lab_ps[:], iota_c,
                        lpt[:, h * CK : (h + 1) * CK],
                        Alu.is_equal, Alu.mult,
                    )
                    for z in range(CK // 128):
                        nc.tensor.matmul(
                            numacc_a[:],
                            msk[:, z * 128 : (z + 1) * 128],
                            ones128[:],
                            start=(nmm[0] == 0), stop=(nmm[0] == N_MM_TOTAL - 1),
                            skip_group_check=True,
                        )
                        nmm[0] += 1
                    if h in hooks:
                        hooks[h]()

            # persistent buffers
            # v_sb[pi, 4*i + n]: n=0 S0, n=1 S1, n=2 p
            v_sb = spool.tile([128, 4 * NI], F32)
            sm0 = spool.tile([128, NI], F32)
            sm1 = spool.tile([128, NI], F32)
            a_t = spool.tile([128, NI], F32)
            b_t = spool.tile([128, NI], F32)
            # composed 2x2 chunk entries (real space), [128, (e, NCH)]
            praw = spool.tile([128, 4 * NCH], F32)

            # ---------------- streaming main loop (halves) ----------------
            def den_half(hh, y16):
                # ---- denominator: S-matmuls into scan-layout PSUM ----
                s_ps = psS.tile([128, 4 * NIH], F32)
                for j in range(NIH):
                    nc.tensor.matmul(
                        s_ps[:, 4 * j : 4 * j + 4],
                        y16[:, j * 128 : (j + 1) * 128],
                        w4,
                        start=True, stop=True,
                    )
                nc.scalar.copy(
                    v_sb[:, hh * 4 * NIH : (hh + 1) * 4 * NIH], s_ps[:]
                )

            def scan_tail(hh):
                # ---- per-half scan prep + chunk compose (Pool) ----
                sl = slice(hh * NIH, (hh + 1) * NIH)
                v3 = v_sb[:].rearrange("p (i n) -> p i n", n=4)[:, sl, :]
                w32q, w32cq = w32[:, sl], w32c[:, sl]
                nc.gpsimd.tensor_tensor(sm0[:, sl], v3[:, :, 0:1], w32q, Alu.mult)
                nc.gpsimd.tensor_tensor(sm0[:, sl], sm0[:, sl], w32cq, Alu.add)
                nc.gpsimd.tensor_tensor(sm1[:, sl], v3[:, :, 1:2], w32q, Alu.mult)
                nc.gpsimd.tensor_scalar(a_t[:, sl], v3[:, :, 2:3], e_b[:, 0:1], None, Alu.mult)
                nc.gpsimd.tensor_tensor(a_t[:, sl], a_t[:, sl], w32q, Alu.mult)
                nc.gpsimd.tensor_scalar(b_t[:, sl], v3[:, :, 2:3], e_b[:, 1:2], None, Alu.mult)
                nc.gpsimd.tensor_tensor(b_t[:, sl], b_t[:, sl], w32q, Alu.mult)
                nc.gpsimd.tensor_tensor(b_t[:, sl], b_t[:, sl], w32cq, Alu.add)

                def tslice(tl, t):
                    return tl[:, sl].rearrange("p (c l) -> p c l", l=LCH)[:, :, t : t + 1]

                # chunk compose (LCH=2): P = M(t1) o M(t0), M(t0) read in place
                qc = slice(hh * NCHH, (hh + 1) * NCHH)
                pb = praw[:].rearrange("p (e c) -> p e c", e=4)
                s0t, s1t = tslice(sm0, 1), tslice(sm1, 1)
                att, btt = tslice(a_t, 1), tslice(b_t, 1)
                for ci, col in enumerate(("0", "1")):
                    pc0 = tslice(sm0 if col == "0" else sm1, 0)
                    pc1 = tslice(a_t if col == "0" else b_t, 0)
                    o0 = pb[:, 0 + ci, qc]        # e for "0"+col
                    o1 = pb[:, 2 + ci, qc]        # e for "1"+col
                    t1 = scan_pool.tile([128, NCHH], F32, tag=f"t1{col}")
                    nc.gpsimd.tensor_tensor(t1[:], s0t, pc0, Alu.mult)
                    nc.gpsimd.tensor_tensor(o0, s1t, pc1, Alu.mult)
                    nc.gpsimd.tensor_tensor(o0, t1[:], o0, Alu.add)
                    t2 = scan_pool.tile([128, NCHH], F32, tag=f"t2{col}")
                    nc.gpsimd.tensor_tensor(t2[:], att, pc0, Alu.mult)
                    nc.gpsimd.tensor_tensor(o1, btt, pc1, Alu.mult)
                    nc.gpsimd.tensor_tensor(o1, t2[:], o1, Alu.add)

                # ship this half's raw chunk matrices
                nc.sync.dma_start(
                    chunkraw_d.ap()[:, :, hh * NCHH : (hh + 1) * NCHH],
                    pb[:, :, qc],
                )

            def full_den(hh, y16):
                den_half(hh, y16)
                scan_tail(hh)

            y16s = [None] * NH
            for hh in range(NH):
                lpt = lpts[hh]
                # exp on ACT; the consumer S-matmuls for half hh are deferred
                # into half hh+1's numerator stream so their weight-load wait
                # can never head-of-line-block the PE label-broadcast chain
                y16 = y_pool.tile([128, GH], FP16)
                nc.scalar.activation(y16[:], lpt[:], Act.Exp)
                y16s[hh] = y16
                hooks = {}
                if hh >= 1:
                    hooks[0] = (lambda p=hh - 1: full_den(p, y16s[p]))
                if hh == NH - 1:
                    hooks[2] = (lambda p=hh: full_den(p, y16s[p]))
                num_stream(hh, lpt, hooks)

            nc.scalar.copy(numpart[:], numacc_a[:])
            nc.scalar.dma_start(numpart_d.ap(), numpart[:])

    nc.compile()
    return nc


_NC_CACHE = None


def _get_program():
    global _NC_CACHE
    if _NC_CACHE is None:
        _NC_CACHE = _build_program()
    return _NC_CACHE


def _den_consts(den_scores):
    """Host O(L) prep: per-state softmax of den_scores -> U columns, e00/e11,
    final arc score."""
    d = den_scores.astype(np.float64)
    s0 = d[: L + 3] - _lse(d[: L + 3], axis=0)   # state-0 arcs (incl final)
    s1 = d[L + 3 :] - _lse(d[L + 3 :], axis=0)   # state-1 arcs
    u0 = np.zeros(C); u1 = np.zeros(C)
    u0[1] = np.exp(s0[0])                        # 'O' arc
    u0[3:128] = np.exp(s0[1:126])                # label arcs
    u1[3:128] = np.exp(s1[1:126])
    e00 = np.exp(s0[126])                        # 0 -> 1 'I-' arc
    e11 = np.exp(s1[0])                          # 1 -> 1 'I-' self loop
    fs = s0[127]                                 # final arc 0 -> 2
    return u0, u1, e00, e11, fs


def _make_in_maps(log_probs, den_scores, input_lens, labels):
    fp16 = np.float16
    pids = np.arange(128)
    u0, u1, e00, e11, _ = _den_consts(den_scores)

    tw = np.zeros((128, TW), dtype=np.float32)
    tw[:, 0] = pids                              # iota_c
    tw[:, 1] = u0
    tw[:, 2] = u1
    tw[:, 3] = (pids == 2)                       # p = y[2] extractor
    tw16 = tw.astype(fp16)

    # scan masks: w32 = 32*valid, w32c = 32*(1-valid); valid = i < len - offs
    offs = ((pids % 16) * NI)[:, None]           # [128, 1]
    iota = np.arange(NI)[None, :]                # [1, NI]

    tmask = np.arange(T)[None, :] < input_lens[:, None]   # [B, T] valid

    in_maps = []
    for k in range(NCORES):
        sl = slice(k * BSH, (k + 1) * BSH)
        lens_p = input_lens[sl][pids // 16][:, None]      # [128, 1]
        valid = (iota < (lens_p - offs)).astype(np.float32)
        cpk = np.zeros((128, CPK), dtype=np.float32)
        cpk[:, O_W32:O_W32 + NI] = SCALE * valid
        cpk[:, O_W32C:O_W32C + NI] = SCALE * (1.0 - valid)
        cpk[:, O_EB] = e00
        cpk[:, O_EB + 1] = e11
        # bt-row p = s*16 + toff holds t = toff*256 + i; lpT column g = i*128 + p
        lp_bt = log_probs[sl].reshape(BSH, 16, NI, C)       # [s, toff, i, c]
        lpt = np.ascontiguousarray(
            lp_bt.transpose(3, 2, 0, 1).reshape(C, BT)      # [c, (i, s, toff)]
        ).astype(fp16)
        lab_bt = labels[sl].reshape(BSH, 16, NI).astype(np.float32)
        lab_bt = np.where(tmask[sl].reshape(BSH, 16, NI), lab_bt, 200.0)
        labt = np.ascontiguousarray(
            lab_bt.transpose(2, 0, 1).reshape(1, BT)
        ).astype(fp16)
        in_maps.append(dict(lpt=lpt, labt=labt, tw=tw16, cpk=cpk))
    return in_maps


def _combine_host(results, den_scores):
    """Fold per-core device outputs into the scalar loss (float64 host fold)."""
    num = 0.0
    logM_all = []  # [64, NCHUNKS_TOTAL, 2, 2] in global sequence order
    corr = LCH * np.log(SCALE)
    fs = _den_consts(den_scores)[4]
    with np.errstate(divide="ignore"):
        for res in results:
            num += float(res["numpart"].sum(dtype=np.float64))
            cl = np.log(res["chunkraw"].astype(np.float64))  # [128, 4, NCH]
            # partition p -> (seq_local = p//16, toff = p%16); chunks (toff, c)
            cl = cl.reshape(BSH, 16, 4, NCH)
            cl = np.transpose(cl, (0, 1, 3, 2)).reshape(BSH, 16 * NCH, 2, 2)
            logM_all.append(cl - corr)
    mats = np.concatenate(logM_all, axis=0)  # [64, 512, 2, 2]

    def compose(Bm, Am):
        # C = B o A : C[i,j] = LSE_k(B[i,k] + A[k,j])
        s = Bm[..., :, :, None] + Am[..., None, :, :]  # [..., i, k, j]
        return _lse(s, axis=-2)

    while mats.shape[1] > 1:
        n = mats.shape[1]
        if n % 2:
            last = mats[:, -1:]
            mats = compose(mats[:, 1::2], mats[:, 0:-1:2])
            mats = np.concatenate([mats, last], axis=1)
        else:
            mats = compose(mats[:, 1::2], mats[:, 0::2])
    den = float(mats[:, 0, 0, 0].sum()) + B * fs
    return np.float32(num - den)


def _lse(x, axis):
    with np.errstate(divide="ignore"):
        m = np.max(x, axis=axis, keepdims=True)
        m = np.where(np.isfinite(m), m, 0.0)
        out = np.squeeze(m, axis) + np.log(
            np.sum(np.exp(x - m), axis=axis)
        )
    return out


def kernel(log_probs, den_scores, input_lens, labels):
    nc = _get_program()
    den_scores = np.asarray(den_scores)
    in_maps = _make_in_maps(
        np.asarray(log_probs), den_scores,
        np.asarray(input_lens), np.asarray(labels),
    )
    res = run_bass_kernel_spmd(nc, in_maps, core_ids=list(range(NCORES)))
    return _combine_host(res.results, den_scores)


# revision 46
# speedup vs baseline: 1.0554x; 1.0554x over previous
"""CRF loss (2-state FSA) on 8 Trainium2 NeuronCores — transposed streaming.

Math: with y = exp(log_probs), the per-step denominator scores are linear in
y:  S0 = sum_c y[c]*U0[c];  S1 = sum_c y[c]*U1[c];  p = y[2]
where U0/U1 are the per-state softmax of den_scores mapped through the arc
table (O(L) host prep, like the scan masks and final-arc score). The 2-state
forward recurrence runs in REAL space as products of 2x2 matrices
  M_t = [[S0, S1], [p*e00, p*e11]]
composed on-device over chunks of LCH=2 steps (scaled by 32 per step against
fp32 underflow). Steps past input_len become 32*I. Raw chunk matrices ship
per half-eighth; the host takes logs in f64, folds the per-sequence chains
with LSE, and sums.

Layout: host ships lp TRANSPOSED as lpT[c, g] fp16 (half the f32 DMA bytes;
~64x less denominator rounding bias than bf16) with column order
g = i*128 + pi, where pi = (seq_local*16 + toff) is the scan partition and
i the within-partition step (t = toff*256 + i). Then:
  - exp on ACT gives y16T in the same layout;
  - S0/S1/p for scan column i = one tiny PE matmul y16T[:, i-block].T @ W
    (W columns: U0, U1, onehot(2), 0) accumulating straight into
    scan-layout PSUM — one [128, 4*NIH] drain per eighth;
  - numerator: PE broadcasts labels (ones[1,128].T @ labT) into PSUM
    (software-pipelined one chunk ahead), one fused DVE STT per 1024-column
    chunk computes (lab==iota_c)*lpT, and PE ones-matmuls accumulate the
    masked values into a [128,1] PSUM (exact fp16 gather, fp32 accumulate).

The 32-chunk DVE STT stream (~38 us) is the critical path: startup DMAs are
ordered so the first STT fires ASAP (lp/label prefixes before bulk loads),
S-matmuls for each eighth are deferred into the next eighth's numerator
stream so their exp-wait cannot head-of-line-block PE's label broadcasts,
and scan prep/compose (Pool) + chunk shipping overlap the stream per eighth
so the only tail is the numerator drain.

Sharding: data-parallel over batch; core k owns sequences [8k, 8k+8).
"""

import os
import sys

import numpy as np

for _p in ("/opt/trn_rl_repo", os.path.expanduser("~/.axon_site/_ro/trn_rl_repo")):
    if os.path.isdir(_p) and _p not in sys.path:
        sys.path.insert(0, _p)

import concourse.bacc as bacc
import concourse.bass as bass
import concourse.mybir as mybir
import concourse.tile as tile
from concourse.bass_utils import run_bass_kernel_spmd

F32 = mybir.dt.float32
FP16 = mybir.dt.float16
Alu = mybir.AluOpType
Act = mybir.ActivationFunctionType

L = 125
C = 128          # symbol classes
B, T = 64, 4096
NCORES = 8
BSH = B // NCORES            # sequences per core = 8
BT = BSH * T                 # positions per core = 32768
NI = BT // 128               # steps per scan partition = 256
NH = 8                       # half-quarters (DMA/exp granularity)
GH = BT // NH                # positions per half = 4096
NIH = NI // NH               # scan steps per half = 32
LCH = 1                      # scan chunk length (raw steps; host composes)
NCH = NI // LCH              # 256 chunk matrices per partition
NCHH = NIH // LCH            # chunk matrices per half = 32
SCALE = 32.0                 # per-step scaling against fp32 underflow
CK = 1024                    # numerator STT chunk positions
NCKH = GH // CK              # chunks per half = 4
# late global chunk ids whose mask runs on Pool from host-prebroadcast labels
POOL_CHUNKS = (16, 19, 22, 25, 28)

# tiny fp16 consts: [iota_c | W(4 cols)]
TW = 5
# packed f32 consts: [w32 | w32c | e_b(2 cols) | iota_c]
O_W32, O_W32C, O_EB, O_IOC = 0, NI, 2 * NI, 2 * NI + 2
CPK = 2 * NI + 3


def _build_program():
    nc = bacc.Bacc("TRN2", target_bir_lowering=False, debug=False)

    lpt_d = nc.dram_tensor("lpt", [128, BT], FP16, kind="ExternalInput")
    labt_d = nc.dram_tensor("labt", [1, BT], FP16, kind="ExternalInput")
    labbc_d = nc.dram_tensor(
        "labbc", [128, len(POOL_CHUNKS) * CK], FP16, kind="ExternalInput"
    )
    tw_d = nc.dram_tensor("tw", [128, TW], FP16, kind="ExternalInput")
    cpk_d = nc.dram_tensor("cpk", [128, CPK], F32, kind="ExternalInput")

    chunkraw_d = nc.dram_tensor("chunkraw", [128, 4, NCH], F32, kind="ExternalOutput")
    numpart_d = nc.dram_tensor("numpart", [128, 1], F32, kind="ExternalOutput")

    with tile.TileContext(nc) as tc:
        with (
            tc.tile_pool(name="const", bufs=1) as cpool,
            tc.tile_pool(name="small", bufs=1) as spool,
            tc.tile_pool(name="y16", bufs=2) as y_pool,
            tc.tile_pool(name="msk", bufs=4) as m_pool,
            tc.tile_pool(name="scan", bufs=1) as scan_pool,
            tc.tile_pool(name="psS", bufs=2, space=bass.MemorySpace.PSUM) as psS,
            tc.tile_pool(name="psL", bufs=2, space=bass.MemorySpace.PSUM) as psL,
            tc.tile_pool(name="psN", bufs=1, space=bass.MemorySpace.PSUM) as psN,
        ):
            # ------- startup loads, ordered for earliest first STT; all
            # load DMAs are issued up-front so later output stores can never
            # block them in the SP queue -------
            lpts = []
            for _hh in range(NH):
                _t = cpool.tile([128, GH], FP16, tag=f"lpt{_hh}")
                lpts.append(_t)
            nc.sync.dma_start(lpts[0][:, 0:CK], lpt_d.ap()[:, 0:CK])
            tw = cpool.tile([128, TW], FP16)
            nc.sync.dma_start(tw[:], tw_d.ap())
            labt_a = cpool.tile([1, 2 * GH], FP16)
            nc.sync.dma_start(labt_a[:], labt_d.ap()[:, 0 : 2 * GH])
            # warmup pieces sized so the DVE stream never starves
            PIECES = [(0, CK, 2 * CK), (0, 2 * CK, GH),
                      (1, 0, 2 * CK), (1, 2 * CK, GH)]
            for _hh, _a, _b in PIECES:
                nc.sync.dma_start(
                    lpts[_hh][:, _a:_b],
                    lpt_d.ap()[:, _hh * GH + _a : _hh * GH + _b],
                )
            nc.sync.dma_start(
                lpts[2][:, 0 : 2 * CK], lpt_d.ap()[:, 2 * GH : 2 * GH + 2 * CK]
            )
            cpk = cpool.tile([128, CPK], F32)
            nc.sync.dma_start(cpk[:], cpk_d.ap())
            labt = cpool.tile([1, BT], FP16)
            nc.sync.dma_start(labt[:], labt_d.ap())
            nc.sync.dma_start(
                lpts[2][:, 2 * CK : GH], lpt_d.ap()[:, 2 * GH + 2 * CK : 3 * GH]
            )
            labbc = cpool.tile([128, len(POOL_CHUNKS) * CK], FP16)
            for _hh in range(3, NH):
                nc.sync.dma_start(
                    lpts[_hh][:], lpt_d.ap()[:, _hh * GH : (_hh + 1) * GH]
                )
                if _hh == 3:
                    nc.sync.dma_start(labbc[:], labbc_d.ap())

            iota_c = tw[:, 0:1]
            w4 = tw[:, 1:5]
            w32 = cpk[:, O_W32:O_W32 + NI]
            w32c = cpk[:, O_W32C:O_W32C + NI]
            e_b = cpk[:, O_EB:O_EB + 2]
            iota_cf = cpk[:, O_IOC:O_IOC + 1]

            ones1 = cpool.tile([1, 128], FP16)
            nc.gpsimd.memset(ones1[:], 1.0)
            ones128 = cpool.tile([128, 1], FP16)
            nc.gpsimd.memset(ones128[:], 1.0)

            numacc_a = psN.tile([128, 1], F32, tag="na")
            numpart = spool.tile([128, 1], F32)
            nmm = [0]
            N_MM_TOTAL = NH * NCKH * (CK // 128)   # incl. deferred pool chunks

            def bcast(hh, h):
                lab_ps = psL.tile([128, CK], F32)
                off = hh * GH + h * CK
                lab_src = labt_a if off < 2 * GH else labt
                for z in range(CK // 512):
                    nc.tensor.matmul(
                        lab_ps[:, z * 512 : (z + 1) * 512],
                        ones1[:],
                        lab_src[:, off + z * 512 : off + (z + 1) * 512],
                        start=True, stop=True,
                    )
                return lab_ps

            DVE_CHUNKS = [k for k in range(NH * NCKH) if k not in POOL_CHUNKS]
            prefetch = [1]           # index into DVE_CHUNKS of next bcast
            lab_q = [bcast(0, 0)]

            pool_oh = {}

            def emit_pool_oh(k):
                # one-hot mask for a Pool-assigned chunk: is_equal on Pool
                # (legal GPSIMD op); the lp multiply happens on DVE at 2x
                j = POOL_CHUNKS.index(k)
                oh = cpool.tile([128, CK], FP16, tag=f"poh{j}")
                nc.gpsimd.tensor_scalar(
                    oh[:], labbc[:, j * CK : (j + 1) * CK], iota_cf, None,
                    Alu.is_equal,
                )
                pool_oh[k] = oh

            def num_stream(hh, lpt, hooks=()):
                for h in range(NCKH):
                    k = hh * NCKH + h
                    if k + 1 in POOL_CHUNKS:
                        emit_pool_oh(k + 1)    # one chunk of Pool lead time
                    if k in POOL_CHUNKS:
                        msk = m_pool.tile([128, CK], FP16)
                        nc.vector.tensor_tensor(
                            msk[:], pool_oh.pop(k)[:],
                            lpt[:, h * CK : (h + 1) * CK], Alu.mult,
                        )
                        for z in range(CK // 128):
                            nc.tensor.matmul(
                                numacc_a[:],
                                msk[:, z * 128 : (z + 1) * 128],
                                ones128[:],
                                start=(nmm[0] == 0),
                                stop=(nmm[0] == N_MM_TOTAL - 1),
                                skip_group_check=True,
                            )
                            nmm[0] += 1
                        if h in hooks:
                            hooks[h]()
                        continue
                    if prefetch[0] < len(DVE_CHUNKS):
                        kn = DVE_CHUNKS[prefetch[0]]
                        lab_q.append(bcast(kn // NCKH, kn % NCKH))
                        prefetch[0] += 1
                    lab_ps = lab_q.pop(0)
                    msk = m_pool.tile([128, CK], FP16)
                    nc.vector.scalar_tensor_tensor(
                        msk[:], lab_ps[:], iota_c,
                        lpt[:, h * CK : (h + 1) * CK],
                        Alu.is_equal, Alu.mult,
                    )
                    for z in range(CK // 128):
                        nc.tensor.matmul(
                            numacc_a[:],
                            msk[:, z * 128 : (z + 1) * 128],
                            ones128[:],
                            start=(nmm[0] == 0), stop=(nmm[0] == N_MM_TOTAL - 1),
                            skip_group_check=True,
                        )
                        nmm[0] += 1
                    if h in hooks:
                        hooks[h]()

            # persistent buffers
            # v_sb[pi, 4*i + n]: n=0 S0, n=1 S1, n=2 p
            v_sb = spool.tile([128, 4 * NI], F32)
            # composed 2x2 chunk entries (real space), [128, (e, NCH)]
            praw = spool.tile([128, 4 * NCH], F32)

            # ---------------- streaming main loop (halves) ----------------
            def den_half(hh, y16):
                # ---- denominator: S-matmuls into scan-layout PSUM ----
                s_ps = psS.tile([128, 4 * NIH], F32)
                for j in range(NIH):
                    nc.tensor.matmul(
                        s_ps[:, 4 * j : 4 * j + 4],
                        y16[:, j * 128 : (j + 1) * 128],
                        w4,
                        start=True, stop=True,
                    )
                nc.scalar.copy(
                    v_sb[:, hh * 4 * NIH : (hh + 1) * 4 * NIH], s_ps[:]
                )

            def scan_tail(hh):
                # per-half scan prep straight into the raw chunk planes;
                # Pool normally, DVE for the last half (idle after its
                # stream) so the tail chain stays short
                e = nc.gpsimd if hh < NH - 1 else nc.vector
                sl = slice(hh * NIH, (hh + 1) * NIH)
                v3 = v_sb[:].rearrange("p (i n) -> p i n", n=4)[:, sl, :]
                w32q, w32cq = w32[:, sl], w32c[:, sl]
                pb = praw[:].rearrange("p (e c) -> p e c", e=4)
                p0, p1 = pb[:, 0, sl], pb[:, 1, sl]
                p2, p3 = pb[:, 2, sl], pb[:, 3, sl]
                e.tensor_tensor(p0, v3[:, :, 0:1], w32q, Alu.mult)
                e.tensor_tensor(p0, p0, w32cq, Alu.add)
                e.tensor_tensor(p1, v3[:, :, 1:2], w32q, Alu.mult)
                e.tensor_scalar(p2, v3[:, :, 2:3], e_b[:, 0:1], None, Alu.mult)
                e.tensor_tensor(p2, p2, w32q, Alu.mult)
                e.tensor_scalar(p3, v3[:, :, 2:3], e_b[:, 1:2], None, Alu.mult)
                e.tensor_tensor(p3, p3, w32q, Alu.mult)
                e.tensor_tensor(p3, p3, w32cq, Alu.add)

                # ship this half's raw step matrices
                nc.sync.dma_start(
                    chunkraw_d.ap()[:, :, hh * NCHH : (hh + 1) * NCHH],
                    pb[:, :, sl],
                )

            def full_den(hh, y16):
                den_half(hh, y16)
                scan_tail(hh)

            y16s = [None] * NH
            for hh in range(NH):
                lpt = lpts[hh]
                # exp on ACT; the consumer S-matmuls for half hh are deferred
                # into half hh+1's numerator stream so their weight-load wait
                # can never head-of-line-block the PE label-broadcast chain
                y16 = y_pool.tile([128, GH], FP16)
                nc.scalar.activation(y16[:], lpt[:], Act.Exp)
                y16s[hh] = y16
                hooks = {}
                if hh >= 1:
                    hooks[0] = (lambda p=hh - 1: full_den(p, y16s[p]))
                if hh == NH - 1:
                    hooks[2] = (lambda p=hh: full_den(p, y16s[p]))
                num_stream(hh, lpt, hooks)

            nc.scalar.copy(numpart[:], numacc_a[:])
            nc.scalar.dma_start(numpart_d.ap(), numpart[:])

    nc.compile()
    return nc


_NC_CACHE = None


def _get_program():
    global _NC_CACHE
    if _NC_CACHE is None:
        _NC_CACHE = _build_program()
    return _NC_CACHE


def _den_consts(den_scores):
    """Host O(L) prep: per-state softmax of den_scores -> U columns, e00/e11,
    final arc score."""
    d = den_scores.astype(np.float64)
    s0 = d[: L + 3] - _lse(d[: L + 3], axis=0)   # state-0 arcs (incl final)
    s1 = d[L + 3 :] - _lse(d[L + 3 :], axis=0)   # state-1 arcs
    u0 = np.zeros(C); u1 = np.zeros(C)
    u0[1] = np.exp(s0[0])                        # 'O' arc
    u0[3:128] = np.exp(s0[1:126])                # label arcs
    u1[3:128] = np.exp(s1[1:126])
    e00 = np.exp(s0[126])                        # 0 -> 1 'I-' arc
    e11 = np.exp(s1[0])                          # 1 -> 1 'I-' self loop
    fs = s0[127]                                 # final arc 0 -> 2
    return u0, u1, e00, e11, fs


def _make_in_maps(log_probs, den_scores, input_lens, labels):
    fp16 = np.float16
    pids = np.arange(128)
    u0, u1, e00, e11, _ = _den_consts(den_scores)

    tw = np.zeros((128, TW), dtype=np.float32)
    tw[:, 0] = pids                              # iota_c
    tw[:, 1] = u0
    tw[:, 2] = u1
    tw[:, 3] = (pids == 2)                       # p = y[2] extractor
    tw16 = tw.astype(fp16)

    # scan masks: w32 = 32*valid, w32c = 32*(1-valid); valid = i < len - offs
    offs = ((pids % 16) * NI)[:, None]           # [128, 1]
    iota = np.arange(NI)[None, :]                # [1, NI]

    tmask = np.arange(T)[None, :] < input_lens[:, None]   # [B, T] valid

    in_maps = []
    for k in range(NCORES):
        sl = slice(k * BSH, (k + 1) * BSH)
        lens_p = input_lens[sl][pids // 16][:, None]      # [128, 1]
        valid = (iota < (lens_p - offs)).astype(np.float32)
        cpk = np.zeros((128, CPK), dtype=np.float32)
        cpk[:, O_W32:O_W32 + NI] = SCALE * valid
        cpk[:, O_W32C:O_W32C + NI] = SCALE * (1.0 - valid)
        cpk[:, O_EB] = e00
        cpk[:, O_EB + 1] = e11
        cpk[:, O_IOC] = pids
        # bt-row p = s*16 + toff holds t = toff*256 + i; lpT column g = i*128 + p
        lp_bt = log_probs[sl].reshape(BSH, 16, NI, C)       # [s, toff, i, c]
        lpt = np.ascontiguousarray(
            lp_bt.transpose(3, 2, 0, 1).reshape(C, BT)      # [c, (i, s, toff)]
        ).astype(fp16)
        lab_bt = labels[sl].reshape(BSH, 16, NI).astype(np.float32)
        lab_bt = np.where(tmask[sl].reshape(BSH, 16, NI), lab_bt, 200.0)
        labt = np.ascontiguousarray(
            lab_bt.transpose(2, 0, 1).reshape(1, BT)
        ).astype(fp16)
        labbc = np.empty((128, len(POOL_CHUNKS) * CK), dtype=fp16)
        for j, kc in enumerate(POOL_CHUNKS):
            labbc[:, j * CK : (j + 1) * CK] = labt[0, kc * CK : (kc + 1) * CK]
        in_maps.append(dict(lpt=lpt, labt=labt, labbc=labbc, tw=tw16, cpk=cpk))
    return in_maps


def _combine_host(results, den_scores):
    """Fold per-core device outputs into the scalar loss (float64 host fold)."""
    num = 0.0
    logM_all = []  # [64, NCHUNKS_TOTAL, 2, 2] in global sequence order
    corr = LCH * np.log(SCALE)
    fs = _den_consts(den_scores)[4]
    with np.errstate(divide="ignore"):
        for res in results:
            num += float(res["numpart"].sum(dtype=np.float64))
            cl = np.log(res["chunkraw"].astype(np.float64))  # [128, 4, NCH]
            # partition p -> (seq_local = p//16, toff = p%16); chunks (toff, c)
            cl = cl.reshape(BSH, 16, 4, NCH)
            cl = np.transpose(cl, (0, 1, 3, 2)).reshape(BSH, 16 * NCH, 2, 2)
            logM_all.append(cl - corr)
    mats = np.concatenate(logM_all, axis=0)  # [64, 512, 2, 2]

    def compose(Bm, Am):
        # C = B o A : C[i,j] = LSE_k(B[i,k] + A[k,j])
        s = Bm[..., :, :, None] + Am[..., None, :, :]  # [..., i, k, j]
        return _lse(s, axis=-2)

    while mats.shape[1] > 1:
        n = mats.shape[1]
        if n % 2:
            last = mats[:, -1:]
            mats = compose(mats[:, 1::2], mats[:, 0:-1:2])
            mats = np.concatenate([mats, last], axis=1)
        else:
            mats = compose(mats[:, 1::2], mats[:, 0::2])
    den = float(mats[:, 0, 0, 0].sum()) + B * fs
    return np.float32(num - den)


def _lse(x, axis):
    with np.errstate(divide="ignore"):
        m = np.max(x, axis=axis, keepdims=True)
        m = np.where(np.isfinite(m), m, 0.0)
        out = np.squeeze(m, axis) + np.log(
            np.sum(np.exp(x - m), axis=axis)
        )
    return out


def kernel(log_probs, den_scores, input_lens, labels):
    nc = _get_program()
    den_scores = np.asarray(den_scores)
    in_maps = _make_in_maps(
        np.asarray(log_probs), den_scores,
        np.asarray(input_lens), np.asarray(labels),
    )
    res = run_bass_kernel_spmd(nc, in_maps, core_ids=list(range(NCORES)))
    return _combine_host(res.results, den_scores)
